# revision 15
# baseline (speedup 1.0000x reference)
"""GAU (gated attention unit) Trainium2 Bass kernel, 8-core SPMD.

Problem: B=4, T=2048, D=1024, DF=2048, S=128, fp32 in/out.
  u = silu(x@Wu+bu); v = silu(x@Wv+bv); z = silu(x@Wqk+bqk)
  q = (z*g0+b0)/sqrt(S); k = z*g1+b1
  scores = (q+u_qk) @ k^T, key-masked by length[b]; attn = softmax
  out = u * (attn@v); y = out@Wo + bo
return y [B,T,D]

Sharding: core c -> batch b=c//2, query half h=c%2 (1024 queries).
k/v are computed for the full batch on both cores of a pair (duplicated)
to avoid collectives (which cost 15us+ constant overhead each).

Length sparsity: keys t >= length[b] are masked to exactly zero
(exp(-1e30) == 0), so all key-side work is restricted to the first
NV = max_b ceil(length[b]/128) chunks of 128 tokens ("key space",
host-compacted and zero-padded).  NV is a compile-time constant; the
compiled program is cached per NV (uniform across the 8 SPMD cores).

Layout strategy (everything stays transposed so no on-device transposes
are needed; host pre-transposes x and pre-packs weights):
  xq   [d, tq]  : query-half tokens   (contract d on partitions)
  xk   [d, tk]  : valid key tokens, compacted, tk = NV*128
  zq/zk[s, t]  = (Wqk lhsT) @ (x rhs)          -> qT [s,tq], kT [s,tk]
  v    [tk, f] = (xk lhsT) @ (Wv rhs)           natural
  uT   [f, tq] = (Wu lhsT) @ (xq rhs)
  pT   [tk,tq] = exp((kT lhsT)@(qT rhs) + mask) softmax numerator
  den  [tq, 1] = (pT lhsT) @ (ones rhs)         per-query denominator
  oT   [f, tq] = (v lhsT) @ (pT rhs), gated *uT
  y    [tq, d] = (oT lhsT) @ (Wo rhs), *1/den, +bo

All matmuls in bf16 with fp32 PSUM accumulation.  silu(x) is computed
as x*sigmoid(x): one Sigmoid activation (scalar engine) + one multiply
(vector engine) -- bit-identical math, and keeps the whole kernel
runnable under CoreSim (which lacks a Silu table).  Softmax skips the
row-max subtraction: pre-softmax logits for this operator are |s| <~ 2
(q is scaled by gamma*0.02-ish weights and 1/sqrt(S)), so exp() cannot
overflow; masked keys get a -1e30 bias -> exp==0 exactly.

A short warm-up burst of throwaway matmuls runs while the first DMAs
land so the tensor engine's clock is already ramped when real work
starts.
"""

import numpy as np
import ml_dtypes

B, T, D, DF, S = 4, 2048, 1024, 2048, 128
TQ = T // 2  # queries per core
N_CORES = 8
BF16 = ml_dtypes.bfloat16

_NC = {}


def _build_nc(nv, with_ubias, with_vbias, with_qkbias, with_obias):
    import concourse.mybir as mybir
    import concourse.tile as tile
    from concourse import bacc
    from concourse.bass import ts, ds

    f32 = mybir.dt.float32
    bf16 = mybir.dt.bfloat16
    AF = mybir.ActivationFunctionType
    OP = mybir.AluOpType

    TK = nv * 128                      # key tokens (compacted, padded)
    NZK = (TK + 511) // 512            # 512-wide column tiles covering TK

    nc = bacc.Bacc("TRN2", dynamic_dma_scratch_size=4096)

    # ---- I/O ----
    # xq is query-chunk-major ([p, qc, kd*512+c]) so zq/u unblock per 1 MB;
    # wu is ft-major ([p, ft, kd*128+c]) so u(ft) streams per 0.25 MB chunk.
    xq_d = nc.dram_tensor("xq", [128, 2, 8 * 512], bf16, kind="ExternalInput")
    xk_d = nc.dram_tensor("xk", [128, 8, TK], bf16, kind="ExternalInput")
    wu_d = nc.dram_tensor("wu", [128, 16, 8 * 128], bf16, kind="ExternalInput")
    wv_d = nc.dram_tensor("wv", [128, 8, DF], bf16, kind="ExternalInput")
    wo_d = nc.dram_tensor("wo", [128, 16, D], bf16, kind="ExternalInput")
    wqk_d = nc.dram_tensor("wqk", [128, 8, S], bf16, kind="ExternalInput")
    qkg_d = nc.dram_tensor("qkg", [128, 4], f32, kind="ExternalInput")
    mask_d = nc.dram_tensor("mask", [128, nv], f32, kind="ExternalInput")
    ones_c_d = nc.dram_tensor("ones_c", [128, 1], bf16, kind="ExternalInput")
    bu_d = nc.dram_tensor("bu", [128, 16], f32, kind="ExternalInput") if with_ubias else None
    bqk_d = nc.dram_tensor("bqk", [128, 1], f32, kind="ExternalInput") if with_qkbias else None
    bv_d = nc.dram_tensor("bv", [1, DF], bf16, kind="ExternalInput") if with_vbias else None
    boe_d = nc.dram_tensor("boe", [128, D], f32, kind="ExternalInput") if with_obias else None
    ones_r_d = nc.dram_tensor("ones_r", [1, 128], bf16, kind="ExternalInput") if with_vbias else None
    y_d = nc.dram_tensor("y", [TQ, D], f32, kind="ExternalOutput")

    with tile.TileContext(nc) as tc:
        with (
            tc.tile_pool(name="res", bufs=1) as res,
            tc.tile_pool(name="bigw", bufs=2) as bigw,
            tc.tile_pool(name="ps", bufs=4, space="PSUM") as psp,
            tc.tile_pool(name="pssc", bufs=3, space="PSUM") as pssc,
            tc.tile_pool(name="psden", bufs=1, space="PSUM") as psden,
        ):
            # ---- resident tiles ----
            v_sb = res.tile([128, nv, DF], bf16)      # [tk%128, tk//128, f]
            uT_sb = res.tile([128, 16, TQ], bf16)     # [f%128, f//128, tq]
            qT_sb = res.tile([128, TQ], bf16)         # [s, tq]
            kT_sb = res.tile([128, TK], bf16)         # [s, tk]
            qkg_sb = res.tile([128, 4], f32)
            mask_sb = res.tile([128, nv], f32)
            ones_c = res.tile([128, 1], bf16)
            inv_sb = res.tile([128, 8], f32)          # 1/den per tq 128-slice
            bu_sb = res.tile([128, 16], f32, name="bu_sb") if with_ubias else None
            bqk_sb = res.tile([128, 1], f32, name="bqk_sb") if with_qkbias else None
            bv_sb = res.tile([1, DF], bf16, name="bv_sb") if with_vbias else None
            boe_sb = res.tile([128, D], f32, name="boe_sb") if with_obias else None
            ones_r = res.tile([1, 128], bf16, name="ones_r") if with_vbias else None

            # big weights rotate through 2 slots: wu, wv, then wo reuses wu's
            wu_sb = bigw.tile([128, 16, 8 * 128], bf16, tag="bigw")
            wv_sb = bigw.tile([128, 8, DF], bf16, tag="bigw")

            with (
                tc.tile_pool(name="proj", bufs=1) as proj,
                tc.tile_pool(name="sg", bufs=3) as sgp,
                tc.tile_pool(name="zf", bufs=2) as zfp,
            ):
                warm_sb = proj.tile([128, 512], bf16, name="warm")
                xq_sb = proj.tile([128, 2, 8 * 512], bf16)
                xk_sb = proj.tile([128, 8, TK], bf16)
                wqk_sb = proj.tile([128, 8, S], bf16)

                # PE warm-up: a dozen matmul passes on a zeroed tile ramp the
                # tensor-engine clock while the first input DMAs stream in.
                nc.vector.memset(warm_sb[:], 0)
                wps = psp.tile([128, 512], f32, tag="ps", name="warmps")
                for i in range(12):
                    nc.tensor.matmul(
                        wps[:], warm_sb[:, 0:128], warm_sb[:],
                        start=(i == 0), stop=(i == 11),
                    )

                # DMA issue order = need order: u(qc0) -> u(qc1) -> zq -> zk -> v.
                # The first GEMM (u qc0 ft0) only needs xq0+wu0 = 1.25 MB.
                if with_ubias:
                    nc.sync.dma_start(bu_sb[:], bu_d[:])
                nc.sync.dma_start(xq_sb[:, 0, :], xq_d[:, 0, :])
                for ft in range(16):
                    nc.sync.dma_start(wu_sb[:, ft, :], wu_d[:, ft, :])
                nc.sync.dma_start(xq_sb[:, 1, :], xq_d[:, 1, :])
                nc.sync.dma_start(wqk_sb[:], wqk_d[:])
                nc.sync.dma_start(qkg_sb[:], qkg_d[:])
                if with_qkbias:
                    nc.sync.dma_start(bqk_sb[:], bqk_d[:])
                nc.sync.dma_start(xk_sb[:], xk_d[:])
                nc.sync.dma_start(wv_sb[:], wv_d[:])
                if with_vbias:
                    nc.sync.dma_start(bv_sb[:], bv_d[:])
                    nc.sync.dma_start(ones_r[:], ones_r_d[:])
                nc.sync.dma_start(mask_sb[:], mask_d[:])
                nc.sync.dma_start(ones_c[:], ones_c_d[:])
                if with_obias:
                    nc.sync.dma_start(boe_sb[:], boe_d[:])

                def silu_affine(ps_ap, out_ap, bias_ap, g_ap, b_ap):
                    # out = (silu(ps + bias)) * g + b   (g,b per-partition)
                    sig = sgp.tile([128, 512], f32, tag="sg")
                    w = ps_ap.shape[-1]
                    if bias_ap is not None:
                        nc.scalar.activation(sig[:, :w], ps_ap, AF.Sigmoid,
                                             bias=bias_ap)
                        raw = zfp.tile([128, 512], f32, tag="zf")
                        nc.vector.tensor_scalar(
                            raw[:, :w], ps_ap, bias_ap, None, OP.add)
                        z = zfp.tile([128, 512], bf16, tag="zf", name="zb")
                        nc.vector.tensor_mul(z[:, :w], raw[:, :w], sig[:, :w])
                    else:
                        nc.scalar.activation(sig[:, :w], ps_ap, AF.Sigmoid)
                        z = zfp.tile([128, 512], bf16, tag="zf", name="zb")
                        nc.vector.tensor_mul(z[:, :w], ps_ap, sig[:, :w])
                    nc.vector.tensor_scalar(
                        out_ap, z[:, :w], g_ap, b_ap, OP.mult, OP.add)

                def silu_to(ps_ap, out_ap, bias_ap=None):
                    # out = silu(ps + bias)
                    sig = sgp.tile([128, 512], f32, tag="sg")
                    w = ps_ap.shape[-1]
                    if bias_ap is not None:
                        nc.scalar.activation(sig[:, :w], ps_ap, AF.Sigmoid,
                                             bias=bias_ap)
                        raw = zfp.tile([128, 512], f32, tag="zf")
                        nc.vector.tensor_scalar(
                            raw[:, :w], ps_ap, bias_ap, None, OP.add)
                        nc.vector.tensor_mul(out_ap, raw[:, :w], sig[:, :w])
                    else:
                        nc.scalar.activation(sig[:, :w], ps_ap, AF.Sigmoid)
                        nc.vector.tensor_mul(out_ap, ps_ap, sig[:, :w])

                def silu_to_expform(ps_ap, out_ap):
                    # out = silu(ps) = ps / (1 + exp(-ps)); uses the Exp act
                    # table so the sigmoid->exp table switch happens early and
                    # hides behind matmuls instead of stalling the first
                    # attention exp.  (biasless variants only)
                    e = sgp.tile([128, 512], f32, tag="sg")
                    w = ps_ap.shape[-1]
                    nc.scalar.activation(e[:, :w], ps_ap, AF.Exp, scale=-1.0)
                    t = zfp.tile([128, 512], f32, tag="zf")
                    nc.vector.tensor_scalar(t[:, :w], e[:, :w], 1.0, None, OP.add)
                    r = sgp.tile([128, 512], f32, tag="sg", name="sgr")
                    nc.vector.reciprocal(r[:, :w], t[:, :w])
                    nc.vector.tensor_mul(out_ap, ps_ap, r[:, :w])

                bqk_ap = bqk_sb[:, 0:1] if with_qkbias else None

                # ---- uT = silu(Wu^T xq + bu)  [f, tq] ----
                for qc in range(2):
                    for ft in range(16):
                        bu_ap = bu_sb[:, ft:ft + 1] if with_ubias else None
                        ps = psp.tile([128, 512], f32, tag="ps")
                        for kd in range(8):
                            nc.tensor.matmul(
                                ps[:], wu_sb[:, ft, ds(kd * 128, 128)],
                                xq_sb[:, qc, ds(kd * 512, 512)],
                                start=(kd == 0), stop=(kd == 7),
                            )
                        silu_to(ps[:], uT_sb[:, ft, ts(qc, 512)], bu_ap)

                # ---- qT = (silu(Wqk^T xq + bqk))*g0 + b0 ----
                for i in range(2):
                    ps = psp.tile([128, 512], f32, tag="ps")
                    for kd in range(8):
                        nc.tensor.matmul(
                            ps[:], wqk_sb[:, kd, :],
                            xq_sb[:, i, ds(kd * 512, 512)],
                            start=(kd == 0), stop=(kd == 7),
                        )
                    silu_affine(ps[:], qT_sb[:, ts(i, 512)], bqk_ap,
                                qkg_sb[:, 0:1], qkg_sb[:, 1:2])

                # ---- kT = (silu(Wqk^T xk + bqk))*g1 + b1 ----
                for i in range(NZK):
                    w = min(512, TK - i * 512)
                    ps = psp.tile([128, 512], f32, tag="ps")
                    for kd in range(8):
                        nc.tensor.matmul(
                            ps[:, :w], wqk_sb[:, kd, :],
                            xk_sb[:, kd, ds(i * 512, w)],
                            start=(kd == 0), stop=(kd == 7),
                        )
                    silu_affine(ps[:, :w], kT_sb[:, ds(i * 512, w)], bqk_ap,
                                qkg_sb[:, 2:3], qkg_sb[:, 3:4])

                # ---- v = silu(xk Wv + bv)  [tk, f] ----
                for j in range(nv):
                    for fc in range(4):
                        ps = psp.tile([128, 512], f32, tag="ps")
                        for kd in range(8):
                            nc.tensor.matmul(
                                ps[:], xk_sb[:, kd, ts(j, 128)],
                                wv_sb[:, kd, ts(fc, 512)],
                                start=(kd == 0),
                                stop=(kd == 7 and not with_vbias),
                            )
                        if with_vbias:
                            nc.tensor.matmul(
                                ps[:], ones_r[0:1, :], bv_sb[0:1, ts(fc, 512)],
                                start=False, stop=True,
                            )
                        if j == nv - 1:
                            silu_to_expform(ps[:], v_sb[:, j, ts(fc, 512)])
                        else:
                            silu_to(ps[:], v_sb[:, j, ts(fc, 512)])

            # wo reuses the wv slot (Tile waits for v matmuls to finish)
            wo_sb = bigw.tile([128, 16, D], bf16, tag="bigw")
            nc.sync.dma_start(wo_sb[:], wo_d[:])

            with (
                tc.tile_pool(name="attn", bufs=1) as attn,
                tc.tile_pool(name="yout", bufs=2) as yout,
            ):
                for qc in range(2):  # tq chunks of 512
                    pT_sb = attn.tile([128, nv, 512], bf16, tag="pT")
                    oT_sb = attn.tile([128, 16, 512], bf16, tag="oT")

                    # scores^T + exp (mask folded in as per-key bias)
                    for j in range(nv):
                        ps = pssc.tile([128, 512], f32, tag="pssc")
                        nc.tensor.matmul(
                            ps[:], kT_sb[:, ts(j, 128)], qT_sb[:, ts(qc, 512)],
                            start=True, stop=True,
                        )
                        nc.scalar.activation(
                            pT_sb[:, j, :], ps[:], AF.Exp,
                            bias=mask_sb[:, j:j + 1],
                        )

                    # oT = (v^T pT) * uT -- ft in groups of 4 so each group's
                    # j-loop starts as soon as exp_j lands (not after the last)
                    for ftg in range(4):
                        pss = [
                            psp.tile([128, 512], f32, tag="ps", name=f"ot_ps{i}")
                            for i in range(4)
                        ]
                        for j in range(nv):
                            for i in range(4):
                                ft = ftg * 4 + i
                                nc.tensor.matmul(
                                    pss[i][:], v_sb[:, j, ts(ft, 128)],
                                    pT_sb[:, j, :],
                                    start=(j == 0), stop=(j == nv - 1),
                                )
                        for i in range(4):
                            ft = ftg * 4 + i
                            nc.vector.tensor_mul(
                                oT_sb[:, ft, :], pss[i][:],
                                uT_sb[:, ft, ts(qc, 512)],
                            )

                    # denominators: den[tq] = sum_tk pT
                    for sl in range(4):
                        dps = psden.tile([128, 1], f32, tag="den")
                        for j in range(nv):
                            nc.tensor.matmul(
                                dps[:], pT_sb[:, j, ts(sl, 128)], ones_c[:, 0:1],
                                start=(j == 0), stop=(j == nv - 1),
                            )
                        nc.vector.reciprocal(
                            inv_sb[:, qc * 4 + sl: qc * 4 + sl + 1], dps[:]
                        )

                    # y = oT^T Wo * inv + bo  (DMA out per 512-col block so the
                    # final store overlaps the last matmuls; the very last
                    # block goes out in 256-col chunks to shorten the tail)
                    for sl in range(4):
                        y_sb = yout.tile([128, D], f32, tag="y")
                        for dc in range(2):
                            ps = psp.tile([128, 512], f32, tag="ps")
                            for ft in range(16):
                                nc.tensor.matmul(
                                    ps[:], oT_sb[:, ft, ts(sl, 128)],
                                    wo_sb[:, ft, ts(dc, 512)],
                                    start=(ft == 0), stop=(ft == 15),
                                )
                            inv_ap = inv_sb[:, qc * 4 + sl: qc * 4 + sl + 1]
                            last = (qc == 1 and sl == 3 and dc == 1)
                            for half in range(2 if last else 1):
                                w = 256 if last else 512
                                sub = ds(dc * 512 + half * 256, w)
                                nc.vector.tensor_scalar(
                                    y_sb[:, sub], ps[:, ds(half * 256, w)],
                                    inv_ap, None, OP.mult,
                                )
                                if with_obias:
                                    nc.vector.tensor_add(
                                        y_sb[:, sub], y_sb[:, sub],
                                        boe_sb[:, sub])
                                nc.sync.dma_start(
                                    y_d[ds(qc * 512 + sl * 128, 128), sub],
                                    y_sb[:, sub],
                                )

    nc.compile()
    return nc


def _get_nc(key):
    if key not in _NC:
        _NC[key] = _build_nc(*key)
    return _NC[key]


def _variant(inputs):
    nv = int(max(
        (int(l) + 127) // 128 for l in np.asarray(inputs["length"]).ravel()
    ))
    nv = max(1, min(nv, 16))
    return (
        nv,
        bool(np.any(np.asarray(inputs["Wu_b"]))),
        bool(np.any(np.asarray(inputs["Wv_b"]))),
        bool(np.any(np.asarray(inputs["Wqk_b"]))),
        bool(np.any(np.asarray(inputs["Wo_b"]))),
    )


def _prep_in_maps(inputs, key):
    nv, with_ubias, with_vbias, with_qkbias, with_obias = key
    TK = nv * 128
    x = np.ascontiguousarray(inputs["x"], dtype=np.float32)
    length = np.asarray(inputs["length"]).astype(np.int64)
    Wu = np.asarray(inputs["Wu_w"], np.float32)
    bu = np.asarray(inputs["Wu_b"], np.float32)
    Wv = np.asarray(inputs["Wv_w"], np.float32)
    bv = np.asarray(inputs["Wv_b"], np.float32)
    Wqk = np.asarray(inputs["Wqk_w"], np.float32)
    bqk = np.asarray(inputs["Wqk_b"], np.float32)
    Wo = np.asarray(inputs["Wo_w"], np.float32)
    bo = np.asarray(inputs["Wo_b"], np.float32)
    gamma = np.asarray(inputs["gamma"], np.float32)
    beta = np.asarray(inputs["beta"], np.float32)
    u_qk = np.asarray(inputs["u_qk"], np.float32)

    inv_s = np.float32(1.0 / np.sqrt(S))
    qkg = np.stack(
        [gamma[0] * inv_s, beta[0] * inv_s + u_qk, gamma[1], beta[1]], axis=1
    ).astype(np.float32)  # [128, 4]

    def pack_w(w, ko):  # [K, N] -> [128, ko, N] (k = o*128 + p)
        return np.ascontiguousarray(
            w.reshape(ko, 128, w.shape[1]).transpose(1, 0, 2).astype(BF16)
        )

    # wu is ft-major: wu_p[p, ft, kd*128+c] = Wu[kd*128+p, ft*128+c]
    wu_p = np.ascontiguousarray(
        Wu.reshape(8, 128, 16, 128).transpose(1, 2, 0, 3)
        .reshape(128, 16, 8 * 128).astype(BF16)
    )
    wv_p = pack_w(Wv, 8)
    wo_p = pack_w(Wo, 16)
    wqk_p = pack_w(Wqk, 8)
    ones_c = np.ones((128, 1), BF16)

    def pack_x(xr):  # [t, D] -> [128, 8, t]  (kd-major planes)
        t = xr.shape[0]
        return np.ascontiguousarray(
            xr.T.astype(BF16).reshape(8, 128, t).transpose(1, 0, 2)
        )

    def pack_xq(xr):  # [1024, D] -> [128, 2, 8*512]  (query-chunk-major)
        # xq_p[p, qc, kd*512+c] = xr[qc*512+c, kd*128+p]
        return np.ascontiguousarray(
            xr.T.astype(BF16).reshape(8, 128, 2, 512).transpose(1, 2, 0, 3)
            .reshape(128, 2, 8 * 512)
        )

    in_maps = []
    for c in range(N_CORES):
        b, h = c // 2, c % 2
        l = int(length[b])
        xb = x[b]  # [T, D]
        xkpad = np.zeros((TK, D), np.float32)
        xkpad[:min(l, TK)] = xb[:min(l, TK)]
        mask = np.where(np.arange(TK) < l, np.float32(0.0), np.float32(-1e30))
        mask_p = np.ascontiguousarray(mask.reshape(nv, 128).T.astype(np.float32))
        m = {
            "xq": pack_xq(xb[h * TQ:(h + 1) * TQ]),
            "xk": pack_x(xkpad),
            "wu": wu_p,
            "wv": wv_p,
            "wo": wo_p,
            "wqk": wqk_p,
            "qkg": qkg,
            "mask": mask_p,
            "ones_c": ones_c,
        }
        if with_ubias:
            m["bu"] = np.ascontiguousarray(bu.reshape(16, 128).T.astype(np.float32))
        if with_qkbias:
            m["bqk"] = np.ascontiguousarray(bqk[:, None].astype(np.float32))
        if with_vbias:
            m["bv"] = np.ascontiguousarray(bv[None, :].astype(BF16))
            m["ones_r"] = np.ones((1, 128), BF16)
        if with_obias:
            m["boe"] = np.ascontiguousarray(
                np.broadcast_to(bo[None, :], (128, D)).astype(np.float32))
        in_maps.append(m)
    return in_maps


def _gather(results):
    y = np.empty((B, T, D), np.float32)
    for c in range(N_CORES):
        b, h = c // 2, c % 2
        y[b, h * TQ:(h + 1) * TQ, :] = results[c]["y"]
    return y


def _run(inputs, trace=False):
    from concourse.bass_utils import run_bass_kernel_spmd

    key = _variant(inputs)
    nc = _get_nc(key)
    in_maps = _prep_in_maps(inputs, key)
    res = run_bass_kernel_spmd(
        nc, in_maps, core_ids=list(range(N_CORES)), trace=trace
    )
    return _gather(res.results), res


def kernel(**inputs) -> np.ndarray:
    out, _ = _run(inputs)
    return out
